# revision 18
# baseline (speedup 1.0000x reference)
"""Trainium2 Bass kernel for the CAP loss (camera-aware proxy memory bank).

Strategy (8 NeuronCores, SPMD, raw Bass engine blocks):
  - The center bank [32000, 2048] is sharded along the center axis (4000
    centers per core) and reordered cam-major on the host: each core holds
    8 slabs of 500 columns (one slab per camera), fp8(e4m3), scaled x32,
    pre-transposed to [128, 16, 512] (cols padded 500->512 for the
    DoubleRow k-pair stride requirement).
  - Samples are sorted by camid on the host; feats are replicated (fp8).
    Per slab g the PE computes only the rows of camera g (DoubleRow fp8
    matmuls, K=2048 accumulated in PSUM) - the intra-camera mask reduces
    useful compute 8x vs the dense [256 x 4000] product. Outputs land at
    PSUM partition base 0; the per-sample exp scale and the accumulator
    are laid out per piece (one column each), so no partition alignment
    with the sample index is needed.
  - The ACT engine applies exp straight out of PSUM with a per-sample
    1/(32*T*||f8||) scale and its fused accum_out produces the per-sample
    partial intra denominators directly. No vector-engine work at all.
  - Everything else is tiny and runs on the host from the SAME quantized
    arrays: the numerator (exact f32), the 8 same-label exps and the
    first-50 hard-negative prefix (<= 66 columns per sample, fp8-dequant
    dots, consistent with the device quantization to ~1e-7).
  - Device output: one [128, 16] f32 tile per core (one column per piece).

Raw Bass (nc.Block) is used instead of the Tile framework: the installed
walrus rejects two raw-ISA instructions Tile's exit barrier emits."""

import numpy as np
import ml_dtypes

from contextlib import ExitStack, contextmanager

import concourse.bass as bass
from concourse import mybir
from concourse.bass_utils import run_bass_kernel_spmd

# problem constants (hardcoded per harness contract)
N, D, M = 256, 2048, 32000
L, C = 4000, 8
T = 0.07
LAMDA = 0.5
NCORES = 8
SHARD = M // NCORES          # 4000 centers per core
CAMW = SHARD // C            # 500 columns per camera per core
CAMP = 512                   # padded slab width (k-pair stride % 16 == 0)
KT = D // 128                # 16 k-tiles
NSLAB = 6                    # slab ring depth
NPSUM = 4                    # psum ring depth
NWARM = 16                   # dummy matmuls to warm the PE clock gate
ACCW = 16                    # fixed accumulator width (>= max piece count)

F32 = mybir.dt.float32
FP8 = mybir.dt.float8e4
DR = mybir.MatmulPerfMode.DoubleRow
CSCALE = 32.0                # host scales centers by 32 before fp8 cast
EXP = mybir.ActivationFunctionType.Exp


@contextmanager
def _lean_block(nc):
    """nc.Block without the end-of-program all-engine event-semaphore
    barrier (~9us of counted epilogue): engines just branch to the end
    block and drain; the runtime completes when every queue retires."""
    nc.check_frozen()
    assert nc.cur_block is None
    blk = bass.BassBlock(nc, f"block_{nc.next_id()}", no_gpsimd_drain=True)
    nc.cur_block = blk
    yield blk
    for engine, last_body in blk.last_body.items():
        with nc.body(last_body, parent=nc.cur_bb, allow_existing_parent=True):
            engine.br(blk.end_bb)
    nc.switch_bb(blk.end_bb)
    gpsimd_type = nc.gpsimd.engine
    for eng_type, eng in nc.engines.items():
        if eng_type == gpsimd_type:
            continue
        d = mybir.InstDrain(
            name=nc.get_next_instruction_name(),
            ins=[], outs=[], bass_is_fusable=False,
        )
        d.engine = eng_type
        eng.add_instruction(d)
    nc.cur_block = None


def _schedule(counts):
    """chunks: cams with samples; pieces[i]: list of (p0, p1) row ranges
    (<=128 wide) of permuted samples for chunk i."""
    offs = np.concatenate([[0], np.cumsum(counts)]).astype(int)
    chunks = [g for g in range(C) if counts[g] > 0]
    pieces = []
    for g in chunks:
        r0, r1 = int(offs[g]), int(offs[g + 1])
        cuts = list(range(r0, r1, 128)) + [r1]
        pieces.append([(cuts[i], cuts[i + 1]) for i in range(len(cuts) - 1)])
    return chunks, pieces


def _build_program(counts) -> bass.Bass:
    chunks, pieces = _schedule(counts)
    nch = len(chunks)
    cum = np.cumsum([len(p) for p in pieces]).astype(int)  # pieces thru chunk
    npieces = int(cum[-1])
    assert npieces <= ACCW

    nc = bass.Bass()
    ctg = nc.dram_tensor("ctg", [C, 128, KT, CAMP], FP8, kind="ExternalInput")
    fTp = nc.dram_tensor("fTp", [128, KT, N], FP8, kind="ExternalInput")
    svd = nc.dram_tensor("svd", [128, ACCW], F32, kind="ExternalInput")
    acc_out = nc.dram_tensor("ACC_out", [128, ACCW], F32, kind="ExternalOutput")

    with ExitStack() as ctx:
        e = ctx.enter_context

        ft_sb = e(nc.sbuf_tensor("ft_sb", [128, KT, N], FP8))
        slabs = [e(nc.sbuf_tensor(f"slab{j}", [128, KT, CAMP], FP8))
                 for j in range(NSLAB)]
        sv_sb = e(nc.sbuf_tensor("sv_sb", [128, ACCW], F32))
        scr = e(nc.sbuf_tensor("scr", [128, CAMW], F32))
        acc = e(nc.sbuf_tensor("acc", [128, ACCW], F32))

        ps = [e(nc.psum_tensor(f"ps{b}", [128, CAMP], F32)) for b in range(NPSUM)]

        sem_ft = e(nc.semaphore("sem_ft"))
        sem_ftb = e(nc.semaphore("sem_ftb"))
        sem_slab = [e(nc.semaphore(f"sem_slab{j}")) for j in range(NSLAB)]
        sem_slab0b = e(nc.semaphore("sem_slab0b"))
        sem_sv = e(nc.semaphore("sem_sv"))
        sem_pe = e(nc.semaphore("sem_pe"))
        sem_act = e(nc.semaphore("sem_act"))
        sem_od = e(nc.semaphore("sem_od"))

        block = e(_lean_block(nc))

        @block.sync
        def _(sync):
            # minimal path to the first matmul: k-halves of feats + slab 0
            g0 = chunks[0]
            sync.dma_start(out=ft_sb[:, 0:8, :], in_=fTp[:, 0:8, :]).then_inc(
                sem_ft, 16)
            sync.dma_start(out=slabs[0][:, 0:8, :],
                           in_=ctg[g0, :, 0:8, :]).then_inc(sem_slab[0], 16)
            sync.dma_start(out=ft_sb[:, 8:16, :], in_=fTp[:, 8:16, :]).then_inc(
                sem_ftb, 16)
            sync.dma_start(out=slabs[0][:, 8:16, :],
                           in_=ctg[g0, :, 8:16, :]).then_inc(sem_slab0b, 16)
            for idx in range(1, nch):
                j = idx % NSLAB
                if idx >= NSLAB:
                    # slot free once PE finished chunk idx-NSLAB
                    sync.wait_ge(sem_pe, int(cum[idx - NSLAB]))
                sync.dma_start(out=slabs[j][:, :, :],
                               in_=ctg[chunks[idx]]).then_inc(sem_slab[j], 16)
            sync.wait_ge(sem_act, npieces)
            sync.dma_start(out=acc_out[:, :], in_=acc[:, :]).then_inc(sem_od, 16)
            sync.wait_ge(sem_od, 16)

        @block.tensor
        def _(tensor):
            tensor.wait_ge(sem_ft, 16)
            # dummy matmuls on the already-loaded ft half: warms the PE clock
            # gate (HAM) while the first center slab is still in flight
            for w in range(NWARM):
                tensor.matmul(ps[NPSUM - 1][:, 0:128], ft_sb[:, 0:2, 0:128],
                              ft_sb[:, 0:2, 0:128], start=True, stop=True,
                              perf_mode=DR)
            slot_seen = [0] * NSLAB
            pc = 0                          # global piece counter
            for idx in range(nch):
                j = idx % NSLAB
                if idx == 0:
                    tensor.wait_ge(sem_slab[0], 16)   # first k-half only
                    slot_seen[0] = 16
                else:
                    slot_seen[j] += 16
                    tensor.wait_ge(sem_slab[j], slot_seen[j])
                for pi, (p0, p1) in enumerate(pieces[idx]):
                    b = pc % NPSUM
                    if pc >= NPSUM:
                        # psum slot free once ACT consumed piece pc-NPSUM
                        tensor.wait_ge(sem_act, pc - NPSUM + 1)
                    for ki in range(0, KT, 2):
                        if idx == 0 and pi == 0 and ki == 8:
                            tensor.wait_ge(sem_ftb, 16)
                            tensor.wait_ge(sem_slab0b, 16)
                        last = tensor.matmul(
                            ps[b][0:p1 - p0, 0:CAMP],
                            ft_sb[:, ki:ki + 2, p0:p1],
                            slabs[j][:, ki:ki + 2, 0:CAMP],
                            start=(ki == 0), stop=(ki == KT - 2),
                            perf_mode=DR)
                    last.then_inc(sem_pe, 1)
                    pc += 1

        @block.scalar
        def _(scalar):
            # per-sample exp scale rides the ACT engine's own HW-DGE ring
            scalar.dma_start(out=sv_sb[:, :], in_=svd[:, :]).then_inc(sem_sv, 16)
            scalar.wait_ge(sem_sv, 16)
            # exp straight out of PSUM; fused accum_out produces the
            # per-sample partial intra denominator for this camera slab
            pc = 0
            for idx in range(nch):
                for (p0, p1) in pieces[idx]:
                    n = p1 - p0
                    scalar.wait_ge(sem_pe, pc + 1)
                    scalar.activation(
                        out=scr[0:n, 0:CAMW],
                        in_=ps[pc % NPSUM][0:n, 0:CAMW],
                        func=EXP, scale=sv_sb[0:n, pc:pc + 1],
                        accum_out=acc[0:n, pc:pc + 1]
                    ).then_inc(sem_act, 1)
                    pc += 1

    return nc


_PROGRAM_CACHE: dict[tuple, bass.Bass] = {}


def _program(counts) -> bass.Bass:
    key = tuple(int(x) for x in counts)
    if key not in _PROGRAM_CACHE:
        _PROGRAM_CACHE[key] = _build_program(counts)
    return _PROGRAM_CACHE[key]


F8 = ml_dtypes.float8_e4m3


def _make_in_maps(feats_p, centers, counts):
    # replicated: fp8 feats (transposed, k-tiled) + per-sample exp scales
    fT = np.ascontiguousarray(feats_p.T).astype(F8)     # [2048, 256]
    fTp = np.ascontiguousarray(
        fT.reshape(KT, 128, N).transpose(1, 0, 2))      # [128, 16, 256]
    fq = fT.astype(np.float32).T                        # dequantized [256, 2048]
    nrm8 = np.linalg.norm(fq, axis=1)                   # ||f8|| per sample
    sv = (1.0 / (CSCALE * T * nrm8)).astype(np.float32)
    _, pieces = _schedule(counts)
    flat = [p for ch in pieces for p in ch]
    svd = np.zeros((128, ACCW), np.float32)
    for q, (p0, p1) in enumerate(flat):
        svd[0:p1 - p0, q] = sv[p0:p1]

    cq = np.ascontiguousarray(centers.T * CSCALE).astype(F8)  # [2048, 32000]
    in_maps = []
    for c in range(NCORES):
        shard = cq[:, c * SHARD:(c + 1) * SHARD]        # [2048, 4000]
        # cam-major: [2048, 500, 8] -> per cam [128, KT, 512] (padded)
        ctg = np.zeros((C, 128, KT, CAMP), F8)
        by_cam = shard.reshape(D, CAMW, C)
        for g in range(C):
            cg = by_cam[:, :, g].reshape(KT, 128, CAMW).transpose(1, 0, 2)
            ctg[g, :, :, 0:CAMW] = cg
        in_maps.append({"ctg": ctg, "fTp": fTp, "svd": svd})
    return in_maps, fq, sv, flat


def _host_tail(results, fq, sv, flat, feats_p, centers, labels_p, camids_p,
               epoch):
    n = labels_p.shape[0]
    denom_intra = np.zeros(n, np.float32)
    accs = [r["ACC_out"] for r in results]
    for q, (p0, p1) in enumerate(flat):
        part = np.zeros(p1 - p0, np.float32)
        for a in accs:
            part += a[0:p1 - p0, q]
        denom_intra[p0:p1] = part

    # same-label exps + first-50 hard negatives, from the SAME quantized
    # arrays the device used (fp8-dequant f32 dots == PE fp8 matmul)
    def cq_cols(cols):
        return (centers[cols] * CSCALE).astype(F8).astype(np.float32)

    lbl_cols = (labels_p[:, None] * C + np.arange(C)[None, :]).reshape(-1)
    cql = cq_cols(lbl_cols).reshape(n, C, D)            # [n, 8, 2048]
    s_lbl = np.einsum('nrd,nd->nr', cql, fq) * sv[:, None]
    B = np.exp(s_lbl).sum(axis=1)
    cqh = cq_cols(np.arange(58))                        # [58, 2048]
    s_head = (fq @ cqh.T) * sv[:, None]
    eh = np.exp(s_head)
    p50 = eh[:, 0:50].sum(axis=1)
    p58 = eh[:, 0:58].sum(axis=1)
    hard = np.where(labels_p <= 6, p58 - B, p50)
    denom_inter = B + hard

    # exact f32 numerator
    own_centers = centers[labels_p * C + camids_p]
    nrm = np.linalg.norm(feats_p, axis=1)
    own = np.einsum('nd,nd->n', feats_p, own_centers) / (T * nrm)

    loss_i = own - np.log(denom_intra)
    loss_j = own - np.log(denom_inter)

    cam_sums = np.zeros(C, np.float32)
    cam_cnts = np.zeros(C, np.float32)
    np.add.at(cam_sums, camids_p, loss_i)
    np.add.at(cam_cnts, camids_p, 1.0)
    loss_intra = -np.sum(
        np.where(cam_cnts > 0, cam_sums / np.maximum(cam_cnts, 1.0), 0.0),
        dtype=np.float32)

    lbl_sums = np.zeros(L, np.float32)
    lbl_cnts = np.zeros(L, np.float32)
    np.add.at(lbl_sums, labels_p, loss_j)
    np.add.at(lbl_cnts, labels_p, 1.0)
    loss_inter = -np.sum(
        np.where(lbl_cnts > 0, lbl_sums / np.maximum(lbl_cnts, 1.0), 0.0),
        dtype=np.float32)

    if int(epoch) < 5:
        return np.float32(loss_intra)
    return np.stack([loss_intra, LAMDA * loss_inter]).astype(np.float32)


def kernel(feats, centers, labels, camids, epoch):
    feats = np.ascontiguousarray(np.asarray(feats, dtype=np.float32))
    centers = np.ascontiguousarray(np.asarray(centers, dtype=np.float32))
    labels = np.asarray(labels).astype(np.int64)
    camids = np.asarray(camids).astype(np.int64)

    perm = np.argsort(camids, kind="stable")
    feats_p, labels_p, camids_p = feats[perm], labels[perm], camids[perm]
    counts = np.bincount(camids_p, minlength=C)

    in_maps, fq, sv, flat = _make_in_maps(feats_p, centers, counts)
    res = run_bass_kernel_spmd(_program(counts), in_maps,
                               list(range(NCORES))).results
    return _host_tail(res, fq, sv, flat, feats_p, centers, labels_p,
                      camids_p, epoch)


# revision 20
# speedup vs baseline: 1.0612x; 1.0612x over previous
"""Trainium2 Bass kernel for the CAP loss (camera-aware proxy memory bank).

Strategy (8 NeuronCores, SPMD, raw Bass engine blocks):
  - The center bank [32000, 2048] is sharded along the center axis (4000
    centers per core) and reordered cam-major on the host: each core holds
    8 slabs of 500 columns (one slab per camera), fp8(e4m3), scaled x32,
    pre-transposed to [128, 16, 512] (cols padded 500->512 for the
    DoubleRow k-pair stride requirement).
  - Samples are sorted by camid on the host; feats are replicated (fp8).
    Per slab g the PE computes only the rows of camera g (DoubleRow fp8
    matmuls, K=2048 accumulated in PSUM) - the intra-camera mask reduces
    useful compute 8x vs the dense [256 x 4000] product. Outputs land at
    PSUM partition base 0; the per-sample exp scale and the accumulator
    are laid out per piece (one column each), so no partition alignment
    with the sample index is needed.
  - The ACT engine applies exp straight out of PSUM with a per-sample
    1/(32*T*||f8||) scale and its fused accum_out produces the per-sample
    partial intra denominators directly. No vector-engine work at all.
  - Everything else is tiny and runs on the host from the SAME quantized
    arrays: the numerator (exact f32), the 8 same-label exps and the
    first-50 hard-negative prefix (<= 66 columns per sample, fp8-dequant
    dots, consistent with the device quantization to ~1e-7).
  - Device output: one [128, 16] f32 tile per core (one column per piece).

Raw Bass (nc.Block) is used instead of the Tile framework: the installed
walrus rejects two raw-ISA instructions Tile's exit barrier emits."""

import numpy as np
import ml_dtypes

from contextlib import ExitStack, contextmanager

import concourse.bass as bass
from concourse import mybir
from concourse.bass_utils import run_bass_kernel_spmd

# problem constants (hardcoded per harness contract)
N, D, M = 256, 2048, 32000
L, C = 4000, 8
T = 0.07
LAMDA = 0.5
NCORES = 8
SHARD = M // NCORES          # 4000 centers per core
CAMW = SHARD // C            # 500 columns per camera per core
CAMP = 512                   # padded slab width (k-pair stride % 16 == 0)
KT = D // 128                # 16 k-tiles
NSLAB = 6                    # slab ring depth
NPSUM = 4                    # psum ring depth
NWARM = 16                   # dummy matmuls to warm the PE clock gate
ACCW = 16                    # fixed accumulator width (>= max piece count)

F32 = mybir.dt.float32
FP8 = mybir.dt.float8e4
DR = mybir.MatmulPerfMode.DoubleRow
CSCALE = 32.0                # host scales centers by 32 before fp8 cast
EXP = mybir.ActivationFunctionType.Exp


@contextmanager
def _lean_block(nc):
    """nc.Block without the end-of-program all-engine event-semaphore
    barrier (~9us of counted epilogue): engines just branch to the end
    block and drain; the runtime completes when every queue retires."""
    nc.check_frozen()
    assert nc.cur_block is None
    blk = bass.BassBlock(nc, f"block_{nc.next_id()}", no_gpsimd_drain=True)
    nc.cur_block = blk
    yield blk
    for engine, last_body in blk.last_body.items():
        with nc.body(last_body, parent=nc.cur_bb, allow_existing_parent=True):
            engine.br(blk.end_bb)
    nc.switch_bb(blk.end_bb)
    gpsimd_type = nc.gpsimd.engine
    for eng_type, eng in nc.engines.items():
        if eng_type == gpsimd_type:
            continue
        d = mybir.InstDrain(
            name=nc.get_next_instruction_name(),
            ins=[], outs=[], bass_is_fusable=False,
        )
        d.engine = eng_type
        eng.add_instruction(d)
    nc.cur_block = None


def _schedule(counts):
    """chunks: cams with samples; pieces[i]: list of (p0, p1) row ranges
    (<=128 wide) of permuted samples for chunk i."""
    offs = np.concatenate([[0], np.cumsum(counts)]).astype(int)
    chunks = [g for g in range(C) if counts[g] > 0]
    pieces = []
    for g in chunks:
        r0, r1 = int(offs[g]), int(offs[g + 1])
        cuts = list(range(r0, r1, 128)) + [r1]
        pieces.append([(cuts[i], cuts[i + 1]) for i in range(len(cuts) - 1)])
    return chunks, pieces


def _build_program(counts) -> bass.Bass:
    chunks, pieces = _schedule(counts)
    nch = len(chunks)
    cum = np.cumsum([len(p) for p in pieces]).astype(int)  # pieces thru chunk
    npieces = int(cum[-1])
    assert npieces <= ACCW

    nc = bass.Bass()
    ctg = nc.dram_tensor("ctg", [C, 128, KT, CAMP], FP8, kind="ExternalInput")
    fTp = nc.dram_tensor("fTp", [128, KT, N], FP8, kind="ExternalInput")
    svd = nc.dram_tensor("svd", [128, ACCW], F32, kind="ExternalInput")
    acc_out = nc.dram_tensor("ACC_out", [128, ACCW], F32, kind="ExternalOutput")

    with ExitStack() as ctx:
        e = ctx.enter_context

        ft_sb = e(nc.sbuf_tensor("ft_sb", [128, KT, N], FP8))
        slabs = [e(nc.sbuf_tensor(f"slab{j}", [128, KT, CAMP], FP8))
                 for j in range(NSLAB)]
        sv_sb = e(nc.sbuf_tensor("sv_sb", [128, ACCW], F32))
        scr = e(nc.sbuf_tensor("scr", [128, CAMW], F32))
        acc = e(nc.sbuf_tensor("acc", [128, ACCW], F32))

        ps = [e(nc.psum_tensor(f"ps{b}", [128, CAMP], F32)) for b in range(NPSUM)]

        sem_ft = e(nc.semaphore("sem_ft"))
        sem_ftb = e(nc.semaphore("sem_ftb"))
        sem_slab = [e(nc.semaphore(f"sem_slab{j}")) for j in range(NSLAB)]
        sem_slab0b = e(nc.semaphore("sem_slab0b"))
        sem_sv = e(nc.semaphore("sem_sv"))
        sem_pe = e(nc.semaphore("sem_pe"))
        sem_act = e(nc.semaphore("sem_act"))
        sem_od = e(nc.semaphore("sem_od"))

        block = e(_lean_block(nc))

        @block.sync
        def _(sync):
            # minimal path to the first matmul: k-halves of feats + slab 0
            # (odd slabs ride the scalar engine's parallel HW-DGE ring)
            g0 = chunks[0]
            sync.dma_start(out=ft_sb[:, 0:8, :], in_=fTp[:, 0:8, :]).then_inc(
                sem_ft, 16)
            sync.dma_start(out=slabs[0][:, 0:8, :],
                           in_=ctg[g0, :, 0:8, :]).then_inc(sem_slab[0], 16)
            for idx in range(2, nch, 2):
                j = idx % NSLAB
                if idx >= NSLAB:
                    # slot free once PE finished chunk idx-NSLAB
                    sync.wait_ge(sem_pe, int(cum[idx - NSLAB]))
                sync.dma_start(out=slabs[j][:, :, :],
                               in_=ctg[chunks[idx]]).then_inc(sem_slab[j], 16)
            sync.wait_ge(sem_od, 16)

        @block.tensor
        def _(tensor):
            tensor.wait_ge(sem_ft, 16)
            # dummy matmuls on the already-loaded ft half: warms the PE clock
            # gate (HAM) while the first center slab is still in flight
            for w in range(NWARM):
                tensor.matmul(ps[NPSUM - 1][:, 0:128], ft_sb[:, 0:2, 0:128],
                              ft_sb[:, 0:2, 0:128], start=True, stop=True,
                              perf_mode=DR)
            slot_seen = [0] * NSLAB
            pc = 0                          # global piece counter
            for idx in range(nch):
                j = idx % NSLAB
                if idx == 0:
                    tensor.wait_ge(sem_slab[0], 16)   # first k-half only
                    slot_seen[0] = 16
                else:
                    slot_seen[j] += 16
                    tensor.wait_ge(sem_slab[j], slot_seen[j])
                for pi, (p0, p1) in enumerate(pieces[idx]):
                    b = pc % NPSUM
                    if pc >= NPSUM:
                        # psum slot free once ACT consumed piece pc-NPSUM
                        tensor.wait_ge(sem_act, pc - NPSUM + 1)
                    for ki in range(0, KT, 2):
                        if idx == 0 and pi == 0 and ki == 8:
                            tensor.wait_ge(sem_ftb, 16)
                            tensor.wait_ge(sem_slab0b, 16)
                        last = tensor.matmul(
                            ps[b][0:p1 - p0, 0:CAMP],
                            ft_sb[:, ki:ki + 2, p0:p1],
                            slabs[j][:, ki:ki + 2, 0:CAMP],
                            start=(ki == 0), stop=(ki == KT - 2),
                            perf_mode=DR)
                    last.then_inc(sem_pe, 1)
                    pc += 1

        @block.scalar
        def _(scalar):
            # setup + odd slabs ride the ACT engine's own HW-DGE ring, in
            # parallel with the sync ring
            scalar.dma_start(out=sv_sb[:, :], in_=svd[:, :]).then_inc(sem_sv, 16)
            g0 = chunks[0]
            scalar.dma_start(out=ft_sb[:, 8:16, :],
                             in_=fTp[:, 8:16, :]).then_inc(sem_ftb, 16)
            scalar.dma_start(out=slabs[0][:, 8:16, :],
                             in_=ctg[g0, :, 8:16, :]).then_inc(sem_slab0b, 16)
            # odd slabs with fresh ring slots issue before any exp work
            for idx in range(1, min(nch, NSLAB), 2):
                scalar.dma_start(out=slabs[idx][:, :, :],
                                 in_=ctg[chunks[idx]]).then_inc(sem_slab[idx], 16)
            scalar.wait_ge(sem_sv, 16)
            # exp straight out of PSUM; fused accum_out produces the
            # per-sample partial intra denominator for this camera slab.
            # Odd slabs needing a recycled slot issue right after the exp
            # that proves PE consumed the old occupant (chunk idx-NSLAB).
            reuse = {}   # piece index after which to issue slab idx
            for idx in range(NSLAB, nch):
                if idx % 2 == 1:
                    reuse[int(cum[idx - NSLAB]) - 1] = idx
            pc = 0
            for idx in range(nch):
                for (p0, p1) in pieces[idx]:
                    n = p1 - p0
                    scalar.wait_ge(sem_pe, pc + 1)
                    scalar.activation(
                        out=scr[0:n, 0:CAMW],
                        in_=ps[pc % NPSUM][0:n, 0:CAMW],
                        func=EXP, scale=sv_sb[0:n, pc:pc + 1],
                        accum_out=acc[0:n, pc:pc + 1]
                    ).then_inc(sem_act, 1)
                    if pc in reuse:
                        jj = reuse[pc] % NSLAB
                        scalar.dma_start(
                            out=slabs[jj][:, :, :],
                            in_=ctg[chunks[reuse[pc]]]).then_inc(sem_slab[jj], 16)
                    pc += 1
            scalar.dma_start(out=acc_out[:, :], in_=acc[:, :]).then_inc(sem_od, 16)

    return nc


_PROGRAM_CACHE: dict[tuple, bass.Bass] = {}


def _program(counts) -> bass.Bass:
    key = tuple(int(x) for x in counts)
    if key not in _PROGRAM_CACHE:
        _PROGRAM_CACHE[key] = _build_program(counts)
    return _PROGRAM_CACHE[key]


F8 = ml_dtypes.float8_e4m3


def _make_in_maps(feats_p, centers, counts):
    # replicated: fp8 feats (transposed, k-tiled) + per-sample exp scales
    fT = np.ascontiguousarray(feats_p.T).astype(F8)     # [2048, 256]
    fTp = np.ascontiguousarray(
        fT.reshape(KT, 128, N).transpose(1, 0, 2))      # [128, 16, 256]
    fq = fT.astype(np.float32).T                        # dequantized [256, 2048]
    nrm8 = np.linalg.norm(fq, axis=1)                   # ||f8|| per sample
    sv = (1.0 / (CSCALE * T * nrm8)).astype(np.float32)
    _, pieces = _schedule(counts)
    flat = [p for ch in pieces for p in ch]
    svd = np.zeros((128, ACCW), np.float32)
    for q, (p0, p1) in enumerate(flat):
        svd[0:p1 - p0, q] = sv[p0:p1]

    cq = np.ascontiguousarray(centers.T * CSCALE).astype(F8)  # [2048, 32000]
    in_maps = []
    for c in range(NCORES):
        shard = cq[:, c * SHARD:(c + 1) * SHARD]        # [2048, 4000]
        # cam-major: [2048, 500, 8] -> per cam [128, KT, 512] (padded)
        ctg = np.zeros((C, 128, KT, CAMP), F8)
        by_cam = shard.reshape(D, CAMW, C)
        for g in range(C):
            cg = by_cam[:, :, g].reshape(KT, 128, CAMW).transpose(1, 0, 2)
            ctg[g, :, :, 0:CAMW] = cg
        in_maps.append({"ctg": ctg, "fTp": fTp, "svd": svd})
    return in_maps, fq, sv, flat


def _host_tail(results, fq, sv, flat, feats_p, centers, labels_p, camids_p,
               epoch):
    n = labels_p.shape[0]
    denom_intra = np.zeros(n, np.float32)
    accs = [r["ACC_out"] for r in results]
    for q, (p0, p1) in enumerate(flat):
        part = np.zeros(p1 - p0, np.float32)
        for a in accs:
            part += a[0:p1 - p0, q]
        denom_intra[p0:p1] = part

    # same-label exps + first-50 hard negatives, from the SAME quantized
    # arrays the device used (fp8-dequant f32 dots == PE fp8 matmul)
    def cq_cols(cols):
        return (centers[cols] * CSCALE).astype(F8).astype(np.float32)

    lbl_cols = (labels_p[:, None] * C + np.arange(C)[None, :]).reshape(-1)
    cql = cq_cols(lbl_cols).reshape(n, C, D)            # [n, 8, 2048]
    s_lbl = np.einsum('nrd,nd->nr', cql, fq) * sv[:, None]
    B = np.exp(s_lbl).sum(axis=1)
    cqh = cq_cols(np.arange(58))                        # [58, 2048]
    s_head = (fq @ cqh.T) * sv[:, None]
    eh = np.exp(s_head)
    p50 = eh[:, 0:50].sum(axis=1)
    p58 = eh[:, 0:58].sum(axis=1)
    hard = np.where(labels_p <= 6, p58 - B, p50)
    denom_inter = B + hard

    # exact f32 numerator
    own_centers = centers[labels_p * C + camids_p]
    nrm = np.linalg.norm(feats_p, axis=1)
    own = np.einsum('nd,nd->n', feats_p, own_centers) / (T * nrm)

    loss_i = own - np.log(denom_intra)
    loss_j = own - np.log(denom_inter)

    cam_sums = np.zeros(C, np.float32)
    cam_cnts = np.zeros(C, np.float32)
    np.add.at(cam_sums, camids_p, loss_i)
    np.add.at(cam_cnts, camids_p, 1.0)
    loss_intra = -np.sum(
        np.where(cam_cnts > 0, cam_sums / np.maximum(cam_cnts, 1.0), 0.0),
        dtype=np.float32)

    lbl_sums = np.zeros(L, np.float32)
    lbl_cnts = np.zeros(L, np.float32)
    np.add.at(lbl_sums, labels_p, loss_j)
    np.add.at(lbl_cnts, labels_p, 1.0)
    loss_inter = -np.sum(
        np.where(lbl_cnts > 0, lbl_sums / np.maximum(lbl_cnts, 1.0), 0.0),
        dtype=np.float32)

    if int(epoch) < 5:
        return np.float32(loss_intra)
    return np.stack([loss_intra, LAMDA * loss_inter]).astype(np.float32)


def kernel(feats, centers, labels, camids, epoch):
    feats = np.ascontiguousarray(np.asarray(feats, dtype=np.float32))
    centers = np.ascontiguousarray(np.asarray(centers, dtype=np.float32))
    labels = np.asarray(labels).astype(np.int64)
    camids = np.asarray(camids).astype(np.int64)

    perm = np.argsort(camids, kind="stable")
    feats_p, labels_p, camids_p = feats[perm], labels[perm], camids[perm]
    counts = np.bincount(camids_p, minlength=C)

    in_maps, fq, sv, flat = _make_in_maps(feats_p, centers, counts)
    res = run_bass_kernel_spmd(_program(counts), in_maps,
                               list(range(NCORES))).results
    return _host_tail(res, fq, sv, flat, feats_p, centers, labels_p,
                      camids_p, epoch)
